# revision 16
# baseline (speedup 1.0000x reference)
"""BinarizedLinear TRN2 kernel v6: y = x @ sign(weight).T + bias.

Full shapes: x [8192, 4096] f32, weight [4096, 4096] f32, bias [4096] f32
-> y [8192, 4096] f32.  Sharding: tokens/4 x out_features/2 over 8 cores;
each core computes a [2048, 2048] block.

All numeric prep happens on host: sign(w) is exact in fp8e4 (values in
{-1,0,+1}), x is pre-cast to an fp8e4 coarse stream plus, for the first
ND=8 of 16 k-chunks (256 wide), an fp8e4 residual stream r = fp8(x -
fp8(x)), giving those chunks ~bf16 accuracy.  Every matmul is an fp8
DoubleRow (K=256 per instruction, 213 ns per 512-col stream), 24 per
[128, 512] psum group; residual matmuls reuse the same sign-weight
tiles.  Host-measured error: max-rel 1.44e-2 / l2-rel 1.87e-2 (gate
2e-2).

Ramp: w chunks are separate tiles (chunk-granular deps) round-robin over
3 DMA queues; 4 dummy matmuls raise the PE p-state while the first
chunks land; strips 0-1 run as a kc-outer wave over 8 psum banks that
consumes each w chunk ~as it arrives, so the PE streams through the
whole w load instead of stalling 40 us on a monolithic-tile dep.
Steady state (strips 2-15) is group-major, x double-buffered 3 strips
ahead.  DVE adds bias out of PSUM; the last group's eviction is split
in half to pipeline the tail drain.
"""
import sys

if "/opt/trn_rl_repo" not in sys.path:
    sys.path.insert(0, "/opt/trn_rl_repo")

import numpy as np
import ml_dtypes
import concourse.bass as bass
import concourse.mybir as mybir
import concourse.tile as tile
from concourse.bass_utils import run_bass_kernel_spmd

TOKENS, IN_F, OUT_F = 8192, 4096, 4096
T_SHARDS, O_SHARDS = 4, 2
TOK_PER = TOKENS // T_SHARDS   # 2048
OUT_PER = OUT_F // O_SHARDS    # 2048
P = 128
TT = TOK_PER // P              # 16 token strips
NCH = IN_F // 256              # 16 k-chunks of 256 (fp8 DoubleRow)
ND = 8                         # chunks 0..7 get a residual stream
NOG = OUT_PER // 512           # 4 psum groups per strip
PREF = 3                       # x strip prefetch depth

F32 = mybir.dt.float32
F8E4 = mybir.dt.float8e4
DR = mybir.MatmulPerfMode.DoubleRow
FP8NP = ml_dtypes.float8_e4m3


def split_excess_waits(nc, max_waits=1):
    """This walrus build encodes at most one semaphore wait per
    instruction; move excess waits onto preceding same-engine NoOps."""
    ctr = 0
    for fn in nc.m.functions:
        for bb in fn.blocks:
            insts = bb.instructions
            i = 0
            while i < len(insts):
                inst = insts[i]
                si = getattr(inst, "sync_info", None)
                ow = list(si.on_wait) if si else []
                if len(ow) > max_waits:
                    extra, keep = ow[:-max_waits], ow[-max_waits:]
                    si.on_wait = keep
                    inst.sync_info = si
                    k = 0
                    for j in range(0, len(extra), max_waits):
                        ctr += 1
                        nop = mybir.InstNoOp(
                            name=f"I-waitsplit-{ctr}", ins=[], outs=[]
                        )
                        nop.engine = inst.engine
                        nop.sync_info = mybir.SyncInfo(
                            on_wait=extra[j : j + max_waits], on_update=[]
                        )
                        insts.insert(i + k, nop)
                        k += 1
                    i += k
                i += 1
    return ctr


def build_nc():
    nc = bass.Bass()
    xq_d = nc.dram_tensor(
        "xq_d", [TT, P, NCH * 2 * P], F8E4, kind="ExternalInput"
    )
    xr_d = nc.dram_tensor(
        "xr_d", [TT, P, ND * 2 * P], F8E4, kind="ExternalInput"
    )
    w_d = nc.dram_tensor(
        "w_d", [NCH, P, 2, OUT_PER], F8E4, kind="ExternalInput"
    )
    biasb = nc.dram_tensor("biasb", [P, OUT_PER], F32, kind="ExternalInput")
    y = nc.dram_tensor("y", [TOK_PER, OUT_PER], F32, kind="ExternalOutput")

    with tile.TileContext(nc) as tc:
        with (
            tc.tile_pool(name="wres", bufs=1) as wres,
            tc.tile_pool(name="xq_p", bufs=PREF + 1) as xq_pool,
            tc.tile_pool(name="xr_p", bufs=PREF + 1) as xr_pool,
            tc.tile_pool(name="outp", bufs=6) as out_pool,
            tc.tile_pool(name="psum", bufs=8, space="PSUM") as psum_pool,
        ):
            wq = [
                wres.tile([P, 2, OUT_PER], F8E4, tag=f"wq{c}", name=f"wq{c}")
                for c in range(NCH)
            ]
            bias_sb = wres.tile([P, OUT_PER], F32, tag="bias")

            x_tiles = {}

            def dma_x(s, qa, qb):
                xq_t = xq_pool.tile([P, NCH, 2, P], F8E4, tag="xq", name="xq")
                xr_t = xr_pool.tile([P, ND, 2, P], F8E4, tag="xr", name="xr")
                qa.dma_start(
                    xq_t[:].rearrange("p c s t -> p (c s t)"), xq_d[s, :, :]
                )
                qb.dma_start(
                    xr_t[:].rearrange("p c s t -> p (c s t)"), xr_d[s, :, :]
                )
                x_tiles[s] = (xq_t, xr_t)

            # ---- front DMA schedule.  Four queues, each ~110 GB/s with
            # ~6-10 us to first arrival; list order = arrival priority.
            # First-needed items are half-split to cut first-arrival
            # latency.  Wave needs chunk c at ~10.7 + 2.76*min(c,8) +
            # 2.56*max(0,c-8) us.
            xq0 = xq_pool.tile([P, NCH, 2, P], F8E4, tag="xq", name="xq")
            xq1 = xq_pool.tile([P, NCH, 2, P], F8E4, tag="xq", name="xq")
            xr0 = xr_pool.tile([P, ND, 2, P], F8E4, tag="xr", name="xr")
            xr1 = xr_pool.tile([P, ND, 2, P], F8E4, tag="xr", name="xr")
            x_tiles[0] = (xq0, xr0)
            x_tiles[1] = (xq1, xr1)

            def half_x(q, t, s, lo, hi):
                q.dma_start(
                    t[:, lo:hi, :, :].rearrange("p c s t -> p (c s t)"),
                    xq_d[s, :, lo * 2 * P : hi * 2 * P],
                )

            # sync:   xq0c0 xq0c1-7 xq1a wq2 wq4 wq5 xq0b wq8 wq11 wq14
            half_x(nc.sync, xq0, 0, 0, 1)
            half_x(nc.sync, xq0, 0, 1, ND)
            half_x(nc.sync, xq1, 1, 0, ND)
            nc.sync.dma_start(wq[2][:], w_d[2])
            nc.sync.dma_start(wq[4][:], w_d[4])
            nc.sync.dma_start(wq[5][:], w_d[5])
            half_x(nc.sync, xq0, 0, ND, NCH)
            nc.sync.dma_start(wq[8][:], w_d[8])
            nc.sync.dma_start(wq[11][:], w_d[11])
            nc.sync.dma_start(wq[14][:], w_d[14])
            # scalar: wq0(og0,og1,og23) wq3 wq6 wq9 wq12 wq15 bias | y
            nc.scalar.dma_start(wq[0][:, :, :512], w_d[0, :, :, :512])
            nc.scalar.dma_start(wq[0][:, :, 512:1024], w_d[0, :, :, 512:1024])
            nc.scalar.dma_start(wq[0][:, :, 1024:], w_d[0, :, :, 1024:])
            for c in (3, 6, 9, 12, 15):
                nc.scalar.dma_start(wq[c][:], w_d[c])
            nc.scalar.dma_start(bias_sb[:], biasb[:])
            # gpsimd: xr0 wq1(2 halves) xq1b xr1 wq7 wq10 wq13 | x strips
            nc.gpsimd.dma_start(
                xr0[:].rearrange("p c s t -> p (c s t)"), xr_d[0, :, :]
            )
            nc.gpsimd.dma_start(wq[1][:, :, :1024], w_d[1, :, :, :1024])
            nc.gpsimd.dma_start(wq[1][:, :, 1024:], w_d[1, :, :, 1024:])
            half_x(nc.gpsimd, xq1, 1, ND, NCH)
            nc.gpsimd.dma_start(
                xr1[:].rearrange("p c s t -> p (c s t)"), xr_d[1, :, :]
            )
            nc.gpsimd.dma_start(wq[7][:], w_d[7])
            nc.gpsimd.dma_start(wq[10][:], w_d[10])
            nc.gpsimd.dma_start(wq[13][:], w_d[13])

            def evict(ps, s, og, split=1, qs=(nc.scalar,)):
                w_ = 512 // split
                for h in range(split):
                    sl_o = slice(og * 512 + h * w_, og * 512 + (h + 1) * w_)
                    sl_p = slice(h * w_, (h + 1) * w_)
                    out_sb = out_pool.tile([P, w_], F32, tag=f"out{w_}",
                                           name="out")
                    nc.vector.tensor_add(
                        out_sb[:], ps[:, sl_p], bias_sb[:, sl_o]
                    )
                    qs[h % len(qs)].dma_start(
                        y[s * P : (s + 1) * P, sl_o], out_sb[:]
                    )

            # ---- ramp wave: strips 0-1, kc-outer across 8 psum banks,
            # consuming each w chunk as it lands.  g = 4*strip + og.
            # For c<2 the w tiles land in two og-halves, so the matmul
            # order follows the halves.
            ps_w = [
                psum_pool.tile([P, 512], F32, tag="ps", name="ps")
                for _ in range(8)
            ]

            def wmm(g, c, resid, start=False, stop=False):
                s, og = divmod(g, NOG)
                xt = x_tiles[s][resid]
                nc.tensor.matmul(
                    ps_w[g][:], xt[:, c, :, :],
                    wq[c][:, :, og * 512 : (og + 1) * 512],
                    start=start, stop=stop, perf_mode=DR,
                )

            for c in range(NCH):
                if c < 2:
                    # og halves 0-1 first (first half-DMA), then 2-3
                    for g in (0, 1):
                        wmm(g, c, 0, start=(c == 0))
                    for g in (0, 1):
                        wmm(g, c, 1)
                    for g in (2, 3):
                        wmm(g, c, 0, start=(c == 0))
                    for g in (2, 3):
                        wmm(g, c, 1)
                    for g in (4, 5, 6, 7):
                        wmm(g, c, 0, start=(c == 0))
                else:
                    for g in (0, 1, 2, 3):
                        wmm(g, c, 0, stop=(c == NCH - 1))
                    if c < ND:  # strip-0 residuals, interleaved early
                        for g in (0, 1, 2, 3):
                            wmm(g, c, 1)
                    for g in (4, 5, 6, 7):
                        wmm(g, c, 0)
                    if c >= ND:  # strip-1 residuals for chunk c-8
                        for g in (4, 5, 6, 7):
                            wmm(g, c - ND, 1, stop=(c == NCH - 1))
            for g in range(8):
                s, og = divmod(g, NOG)
                evict(ps_w[g], s, og)
            x_tiles.pop(0)
            x_tiles.pop(1)

            # ---- steady state: strips 2..15, group-major.
            for s in range(2, PREF + 2):
                dma_x(s, nc.sync if s % 2 == 0 else nc.gpsimd,
                      nc.gpsimd if s % 2 == 0 else nc.sync)
            for s in range(2, TT):
                if s + PREF < TT:
                    sp = s + PREF
                    dma_x(sp, nc.sync if sp % 2 == 0 else nc.gpsimd,
                          nc.gpsimd if sp % 2 == 0 else nc.sync)
                xq_t, xr_t = x_tiles.pop(s)
                last = s == TT - 1
                for og in range(NOG):
                    ps = psum_pool.tile([P, 512], F32, tag="ps", name="ps")
                    sl = slice(og * 512, (og + 1) * 512)
                    for c in range(NCH):
                        nc.tensor.matmul(
                            ps[:], xq_t[:, c, :, :], wq[c][:, :, sl],
                            start=(c == 0), stop=False, perf_mode=DR,
                        )
                    for c in range(ND):
                        nc.tensor.matmul(
                            ps[:], xr_t[:, c, :, :], wq[c][:, :, sl],
                            start=False, stop=(c == ND - 1), perf_mode=DR,
                        )
                    if last and og == NOG - 1:
                        # final group: fine split, drain across the two
                        # HWDGE queues (gpsimd's SWDGE adds ~1us latency)
                        evict(ps, s, og, split=4, qs=(nc.sync, nc.scalar))
                    elif last and og == NOG - 2:
                        evict(ps, s, og, split=2, qs=(nc.scalar, nc.sync))
                    else:
                        evict(ps, s, og)

    split_excess_waits(nc)
    return nc


_NC = None


def _get_nc():
    global _NC
    if _NC is None:
        _NC = build_nc()
    return _NC


def make_in_maps(x, weight, bias):
    x = np.asarray(x, dtype=np.float32)
    weight = np.asarray(weight, dtype=np.float32)
    bias = np.asarray(bias, dtype=np.float32)

    a8 = x.astype(FP8NP)
    r8 = (x[:, : ND * 256] - a8[:, : ND * 256].astype(np.float32)).astype(
        FP8NP
    )
    # [tok, k] -> [th, tt, t(128), c, s, kp(128)] -> [th, tt, kp, c, s, t]
    aq = np.ascontiguousarray(
        a8.reshape(T_SHARDS, TT, P, NCH, 2, P).transpose(0, 1, 5, 3, 4, 2)
    ).reshape(T_SHARDS, TT, P, NCH * 2 * P)
    rq = np.ascontiguousarray(
        r8.reshape(T_SHARDS, TT, P, ND, 2, P).transpose(0, 1, 5, 3, 4, 2)
    ).reshape(T_SHARDS, TT, P, ND * 2 * P)

    S = np.sign(weight).T.astype(FP8NP)  # [in, out], exact in fp8
    in_maps = []
    for core in range(8):
        th, oq = divmod(core, O_SHARDS)
        wsh = S[:, oq * OUT_PER : (oq + 1) * OUT_PER]
        # [in, out] -> w_d[c, p, s, o] = S[c*256 + s*128 + p, o]
        wr = np.ascontiguousarray(
            wsh.reshape(NCH, 2, P, OUT_PER).transpose(0, 2, 1, 3)
        )
        in_maps.append(
            {
                "xq_d": aq[th],
                "xr_d": rq[th],
                "w_d": wr,
                "biasb": np.ascontiguousarray(
                    np.broadcast_to(
                        bias[oq * OUT_PER : (oq + 1) * OUT_PER], (P, OUT_PER)
                    )
                ),
            }
        )
    return in_maps


def assemble(results):
    out = np.empty((TOKENS, OUT_F), np.float32)
    for core in range(8):
        th, oq = divmod(core, O_SHARDS)
        out[
            th * TOK_PER : (th + 1) * TOK_PER,
            oq * OUT_PER : (oq + 1) * OUT_PER,
        ] = results[core]["y"]
    return out


def kernel(x, weight, bias):
    in_maps = make_in_maps(x, weight, bias)
    res = run_bass_kernel_spmd(_get_nc(), in_maps, core_ids=list(range(8)))
    return assemble(res.results)


# revision 20
# speedup vs baseline: 1.0053x; 1.0053x over previous
"""BinarizedLinear TRN2 kernel v6: y = x @ sign(weight).T + bias.

Full shapes: x [8192, 4096] f32, weight [4096, 4096] f32, bias [4096] f32
-> y [8192, 4096] f32.  Sharding: tokens/4 x out_features/2 over 8 cores;
each core computes a [2048, 2048] block.

All numeric prep happens on host: sign(w) is exact in fp8e4 (values in
{-1,0,+1}), x is pre-cast to an fp8e4 coarse stream plus, for the first
ND=8 of 16 k-chunks (256 wide), an fp8e4 residual stream r = fp8(x -
fp8(x)), giving those chunks ~bf16 accuracy.  Every matmul is an fp8
DoubleRow (K=256 per instruction, 213 ns per 512-col stream), 24 per
[128, 512] psum group; residual matmuls reuse the same sign-weight
tiles.  Host-measured error: max-rel 1.44e-2 / l2-rel 1.87e-2 (gate
2e-2).

Ramp: w chunks are separate tiles (chunk-granular deps) round-robin over
3 DMA queues; 4 dummy matmuls raise the PE p-state while the first
chunks land; strips 0-1 run as a kc-outer wave over 8 psum banks that
consumes each w chunk ~as it arrives, so the PE streams through the
whole w load instead of stalling 40 us on a monolithic-tile dep.
Steady state (strips 2-15) is group-major, x double-buffered 3 strips
ahead.  DVE adds bias out of PSUM; the last group's eviction is split
in half to pipeline the tail drain.
"""
import sys

if "/opt/trn_rl_repo" not in sys.path:
    sys.path.insert(0, "/opt/trn_rl_repo")

import numpy as np
import ml_dtypes
import concourse.bass as bass
import concourse.mybir as mybir
import concourse.tile as tile
from concourse.bass_utils import run_bass_kernel_spmd

TOKENS, IN_F, OUT_F = 8192, 4096, 4096
T_SHARDS, O_SHARDS = 4, 2
TOK_PER = TOKENS // T_SHARDS   # 2048
OUT_PER = OUT_F // O_SHARDS    # 2048
P = 128
TT = TOK_PER // P              # 16 token strips
NCH = IN_F // 256              # 16 k-chunks of 256 (fp8 DoubleRow)
ND = 8                         # chunks 0..7 get a residual stream
NOG = OUT_PER // 512           # 4 psum groups per strip
PREF = 3                       # x strip prefetch depth

F32 = mybir.dt.float32
F8E4 = mybir.dt.float8e4
DR = mybir.MatmulPerfMode.DoubleRow
FP8NP = ml_dtypes.float8_e4m3


def split_excess_waits(nc, max_waits=1):
    """This walrus build encodes at most one semaphore wait per
    instruction; move excess waits onto preceding same-engine NoOps."""
    ctr = 0
    for fn in nc.m.functions:
        for bb in fn.blocks:
            insts = bb.instructions
            i = 0
            while i < len(insts):
                inst = insts[i]
                si = getattr(inst, "sync_info", None)
                ow = list(si.on_wait) if si else []
                if len(ow) > max_waits:
                    extra, keep = ow[:-max_waits], ow[-max_waits:]
                    si.on_wait = keep
                    inst.sync_info = si
                    k = 0
                    for j in range(0, len(extra), max_waits):
                        ctr += 1
                        nop = mybir.InstNoOp(
                            name=f"I-waitsplit-{ctr}", ins=[], outs=[]
                        )
                        nop.engine = inst.engine
                        nop.sync_info = mybir.SyncInfo(
                            on_wait=extra[j : j + max_waits], on_update=[]
                        )
                        insts.insert(i + k, nop)
                        k += 1
                    i += k
                i += 1
    return ctr


def build_nc():
    nc = bass.Bass()
    xq_d = nc.dram_tensor(
        "xq_d", [TT, P, NCH * 2 * P], F8E4, kind="ExternalInput"
    )
    xr_d = nc.dram_tensor(
        "xr_d", [TT, P, ND * 2 * P], F8E4, kind="ExternalInput"
    )
    w_d = nc.dram_tensor(
        "w_d", [NCH, P, 2, OUT_PER], F8E4, kind="ExternalInput"
    )
    biasb = nc.dram_tensor("biasb", [P, OUT_PER], F32, kind="ExternalInput")
    y = nc.dram_tensor("y", [TOK_PER, OUT_PER], F32, kind="ExternalOutput")

    with tile.TileContext(nc) as tc:
        with (
            tc.tile_pool(name="wres", bufs=1) as wres,
            tc.tile_pool(name="xq_p", bufs=PREF + 1) as xq_pool,
            tc.tile_pool(name="xr_p", bufs=PREF + 1) as xr_pool,
            tc.tile_pool(name="outp", bufs=6) as out_pool,
            tc.tile_pool(name="psum", bufs=8, space="PSUM") as psum_pool,
        ):
            wq = [
                wres.tile([P, 2, OUT_PER], F8E4, tag=f"wq{c}", name=f"wq{c}")
                for c in range(NCH)
            ]
            bias_sb = wres.tile([P, OUT_PER], F32, tag="bias")
            scratch = wres.tile([P, 2, 512], F8E4, tag="scr")
            # memset first so the DVE queue's later work doesn't delay
            # the PE warm-up matmuls that read this tile.
            nc.vector.memset(scratch[:], 0)

            x_tiles = {}

            def dma_x(s, qa, qb):
                xq_t = xq_pool.tile([P, NCH, 2, P], F8E4, tag="xq", name="xq")
                xr_t = xr_pool.tile([P, ND, 2, P], F8E4, tag="xr", name="xr")
                qa.dma_start(
                    xq_t[:].rearrange("p c s t -> p (c s t)"), xq_d[s, :, :]
                )
                qb.dma_start(
                    xr_t[:].rearrange("p c s t -> p (c s t)"), xr_d[s, :, :]
                )
                x_tiles[s] = (xq_t, xr_t)

            # ---- front DMA schedule.  Four queues, each ~110 GB/s with
            # ~6-10 us to first arrival; list order = arrival priority.
            # First-needed items are half-split to cut first-arrival
            # latency.  Wave needs chunk c at ~10.7 + 2.76*min(c,8) +
            # 2.56*max(0,c-8) us.
            xq0 = xq_pool.tile([P, NCH, 2, P], F8E4, tag="xq", name="xq")
            xq1 = xq_pool.tile([P, NCH, 2, P], F8E4, tag="xq", name="xq")
            xr0 = xr_pool.tile([P, ND, 2, P], F8E4, tag="xr", name="xr")
            xr1 = xr_pool.tile([P, ND, 2, P], F8E4, tag="xr", name="xr")
            x_tiles[0] = (xq0, xr0)
            x_tiles[1] = (xq1, xr1)

            def half_x(q, t, s, lo, hi):
                q.dma_start(
                    t[:, lo:hi, :, :].rearrange("p c s t -> p (c s t)"),
                    xq_d[s, :, lo * 2 * P : hi * 2 * P],
                )

            # sync:   xq0a xq1a wq2 wq4 wq5 xq0b wq8 wq11 wq14 | x strips
            half_x(nc.sync, xq0, 0, 0, ND)
            half_x(nc.sync, xq1, 1, 0, ND)
            nc.sync.dma_start(wq[2][:], w_d[2])
            nc.sync.dma_start(wq[4][:], w_d[4])
            nc.sync.dma_start(wq[5][:], w_d[5])
            half_x(nc.sync, xq0, 0, ND, NCH)
            nc.sync.dma_start(wq[8][:], w_d[8])
            nc.sync.dma_start(wq[11][:], w_d[11])
            nc.sync.dma_start(wq[14][:], w_d[14])
            # scalar: wq0(2 halves) wq3 wq6 wq9 wq12 wq15 bias | y outs
            nc.scalar.dma_start(wq[0][:, :, :1024], w_d[0, :, :, :1024])
            nc.scalar.dma_start(wq[0][:, :, 1024:], w_d[0, :, :, 1024:])
            for c in (3, 6, 9, 12, 15):
                nc.scalar.dma_start(wq[c][:], w_d[c])
            nc.scalar.dma_start(bias_sb[:], biasb[:])
            # gpsimd: xr0 wq1(2 halves) xq1b xr1 wq7 wq10 wq13 | x strips
            nc.gpsimd.dma_start(
                xr0[:].rearrange("p c s t -> p (c s t)"), xr_d[0, :, :]
            )
            nc.gpsimd.dma_start(wq[1][:, :, :1024], w_d[1, :, :, :1024])
            nc.gpsimd.dma_start(wq[1][:, :, 1024:], w_d[1, :, :, 1024:])
            half_x(nc.gpsimd, xq1, 1, ND, NCH)
            nc.gpsimd.dma_start(
                xr1[:].rearrange("p c s t -> p (c s t)"), xr_d[1, :, :]
            )
            nc.gpsimd.dma_start(wq[7][:], w_d[7])
            nc.gpsimd.dma_start(wq[10][:], w_d[10])
            nc.gpsimd.dma_start(wq[13][:], w_d[13])

            # ---- PE p-state warm-up: 2 dummy matmuls on the zeroed tile.
            ps_warm = psum_pool.tile([P, 512], F32, tag="ps", name="psw")
            for _ in range(2):
                nc.tensor.matmul(
                    ps_warm[:], scratch[:, :, :P], scratch[:],
                    start=True, stop=True, perf_mode=DR,
                )

            def evict(ps, s, og, split=1, qs=(nc.scalar,)):
                w_ = 512 // split
                for h in range(split):
                    sl_o = slice(og * 512 + h * w_, og * 512 + (h + 1) * w_)
                    sl_p = slice(h * w_, (h + 1) * w_)
                    out_sb = out_pool.tile([P, w_], F32, tag=f"out{w_}",
                                           name="out")
                    nc.vector.tensor_add(
                        out_sb[:], ps[:, sl_p], bias_sb[:, sl_o]
                    )
                    qs[h % len(qs)].dma_start(
                        y[s * P : (s + 1) * P, sl_o], out_sb[:]
                    )

            # ---- ramp wave: strips 0-1, kc-outer across 8 psum banks,
            # consuming each w chunk as it lands.  g = 4*strip + og.
            # For c<2 the w tiles land in two og-halves, so the matmul
            # order follows the halves.
            ps_w = [
                psum_pool.tile([P, 512], F32, tag="ps", name="ps")
                for _ in range(8)
            ]

            def wmm(g, c, resid, start=False, stop=False):
                s, og = divmod(g, NOG)
                xt = x_tiles[s][resid]
                nc.tensor.matmul(
                    ps_w[g][:], xt[:, c, :, :],
                    wq[c][:, :, og * 512 : (og + 1) * 512],
                    start=start, stop=stop, perf_mode=DR,
                )

            for c in range(NCH):
                if c < 2:
                    # og halves 0-1 first (first half-DMA), then 2-3
                    for g in (0, 1):
                        wmm(g, c, 0, start=(c == 0))
                    for g in (0, 1):
                        wmm(g, c, 1)
                    for g in (2, 3):
                        wmm(g, c, 0, start=(c == 0))
                    for g in (2, 3):
                        wmm(g, c, 1)
                    for g in (4, 5, 6, 7):
                        wmm(g, c, 0, start=(c == 0))
                else:
                    for g in (0, 1, 2, 3):
                        wmm(g, c, 0, stop=(c == NCH - 1))
                    if c < ND:  # strip-0 residuals, interleaved early
                        for g in (0, 1, 2, 3):
                            wmm(g, c, 1)
                    for g in (4, 5, 6, 7):
                        wmm(g, c, 0)
                    if c >= ND:  # strip-1 residuals for chunk c-8
                        for g in (4, 5, 6, 7):
                            wmm(g, c - ND, 1, stop=(c == NCH - 1))
            for g in range(8):
                s, og = divmod(g, NOG)
                evict(ps_w[g], s, og)
            x_tiles.pop(0)
            x_tiles.pop(1)

            # ---- steady state: strips 2..15, group-major.
            for s in range(2, PREF + 2):
                dma_x(s, nc.sync if s % 2 == 0 else nc.gpsimd,
                      nc.gpsimd if s % 2 == 0 else nc.sync)
            for s in range(2, TT):
                if s + PREF < TT:
                    sp = s + PREF
                    dma_x(sp, nc.sync if sp % 2 == 0 else nc.gpsimd,
                          nc.gpsimd if sp % 2 == 0 else nc.sync)
                xq_t, xr_t = x_tiles.pop(s)
                last = s == TT - 1
                for og in range(NOG):
                    ps = psum_pool.tile([P, 512], F32, tag="ps", name="ps")
                    sl = slice(og * 512, (og + 1) * 512)
                    for c in range(NCH):
                        nc.tensor.matmul(
                            ps[:], xq_t[:, c, :, :], wq[c][:, :, sl],
                            start=(c == 0), stop=False, perf_mode=DR,
                        )
                    for c in range(ND):
                        nc.tensor.matmul(
                            ps[:], xr_t[:, c, :, :], wq[c][:, :, sl],
                            start=False, stop=(c == ND - 1), perf_mode=DR,
                        )
                    if last and og == NOG - 1:
                        # final group: split drain across the two HWDGE
                        # queues (gpsimd's SWDGE adds ~1us latency)
                        evict(ps, s, og, split=2, qs=(nc.sync, nc.scalar))
                    elif last and og == NOG - 2:
                        evict(ps, s, og, split=2, qs=(nc.scalar, nc.sync))
                    else:
                        evict(ps, s, og)

    split_excess_waits(nc)
    return nc


_NC = None


def _get_nc():
    global _NC
    if _NC is None:
        _NC = build_nc()
    return _NC


def make_in_maps(x, weight, bias):
    x = np.asarray(x, dtype=np.float32)
    weight = np.asarray(weight, dtype=np.float32)
    bias = np.asarray(bias, dtype=np.float32)

    a8 = x.astype(FP8NP)
    r8 = (x[:, : ND * 256] - a8[:, : ND * 256].astype(np.float32)).astype(
        FP8NP
    )
    # [tok, k] -> [th, tt, t(128), c, s, kp(128)] -> [th, tt, kp, c, s, t]
    aq = np.ascontiguousarray(
        a8.reshape(T_SHARDS, TT, P, NCH, 2, P).transpose(0, 1, 5, 3, 4, 2)
    ).reshape(T_SHARDS, TT, P, NCH * 2 * P)
    rq = np.ascontiguousarray(
        r8.reshape(T_SHARDS, TT, P, ND, 2, P).transpose(0, 1, 5, 3, 4, 2)
    ).reshape(T_SHARDS, TT, P, ND * 2 * P)

    S = np.sign(weight).T.astype(FP8NP)  # [in, out], exact in fp8
    in_maps = []
    for core in range(8):
        th, oq = divmod(core, O_SHARDS)
        wsh = S[:, oq * OUT_PER : (oq + 1) * OUT_PER]
        # [in, out] -> w_d[c, p, s, o] = S[c*256 + s*128 + p, o]
        wr = np.ascontiguousarray(
            wsh.reshape(NCH, 2, P, OUT_PER).transpose(0, 2, 1, 3)
        )
        in_maps.append(
            {
                "xq_d": aq[th],
                "xr_d": rq[th],
                "w_d": wr,
                "biasb": np.ascontiguousarray(
                    np.broadcast_to(
                        bias[oq * OUT_PER : (oq + 1) * OUT_PER], (P, OUT_PER)
                    )
                ),
            }
        )
    return in_maps


def assemble(results):
    out = np.empty((TOKENS, OUT_F), np.float32)
    for core in range(8):
        th, oq = divmod(core, O_SHARDS)
        out[
            th * TOK_PER : (th + 1) * TOK_PER,
            oq * OUT_PER : (oq + 1) * OUT_PER,
        ] = results[core]["y"]
    return out


def kernel(x, weight, bias):
    in_maps = make_in_maps(x, weight, bias)
    res = run_bass_kernel_spmd(_get_nc(), in_maps, core_ids=list(range(8)))
    return assemble(res.results)
